# revision 37
# baseline (speedup 1.0000x reference)
"""DiffAttention Trainium2 kernel (8-core SPMD, full-I/O contract).

Sharding: core c = (batch b = c//4) x (head-group g = c%4, 4 of 16 v-heads).
Each core computes qkv for its heads, two sub-attentions (differential),
softmax via exp + ones-column denominator trick, combine + RMSNorm, and a
partial output projection o @ W_proj[rows].  Host sums the 4 partials per
batch and adds b_proj.

Device layout notes:
  - x is uploaded pre-transposed (xT [1024, 2048]) so both qkv matmul
    orientations need no on-device transposes.
  - scores are computed transposed (S^T [ktok, qtok]) so the AV matmul can
    keep v stationary and stream expS^T; the softmax denominator comes from
    an appended ones column in v (row 64 of the AV accumulator).
  - all matmuls run as float32r (full-rate fp32 PE mode; every moving dim is
    >= 256 so it streams at 1 cycle/row).
  - RMSNorm rstd is computed as exp(-0.5*ln(ms+eps)) so only one ACT table
    set (natural_log_exp) is ever needed; the cross-partition sums for the
    mean-square and the normalizer broadcasts run on GPSIMD
    (partition_all_reduce / partition_broadcast), keeping PSUM free.
  - PSUM: 2-bank score tiles double-buffered (4 banks) + one 4-bank slot
    rotating O1 -> O2 -> proj; the projection of q-block N is emitted inside
    q-block N+1's attention so it never stalls the exp pipeline.
  - W_qkv q-columns are pre-scaled by hd^-0.5, W_proj rows by
    subln_w * (1 - lambda_init) on the host.
  - x/W live in a phase-1 pool that is released before the attention-phase
    buffers are allocated, so peak SBUF stays under budget.
"""

import numpy as np

P = 128
N_TOK = 2048
DIM = 1024
NCORES = 8
HD = 32
VD = 64            # 2*hd, v-head dim
VD1 = VD + 1       # + ones column for the softmax denominator
NKD = DIM // P     # 8 k-tiles over the model dim
NKT = N_TOK // P   # 16 token tiles
QB = 512           # query block
NQB = N_TOK // QB  # 4
LAMBDA_INIT = 0.8 - 0.6 * float(np.exp(-0.3 * 12))
EPS = 1e-5
SCALE = HD ** -0.5

_CACHE: dict = {}


def _build_module():
    from contextlib import ExitStack

    import concourse.bass as bass  # noqa: F401
    import concourse.mybir as mybir
    import concourse.tile as tile
    from concourse import bacc, bass_isa

    f32 = mybir.dt.float32
    f32r = mybir.dt.float32r
    AF = mybir.ActivationFunctionType

    nc = bacc.Bacc(
        "TRN2", target_bir_lowering=False, debug=False, num_devices=NCORES
    )

    xT_d = nc.dram_tensor("xt", [DIM, N_TOK], f32r, kind="ExternalInput").ap()
    wqkv_d = nc.dram_tensor("wqkv", [DIM, 768], f32r, kind="ExternalInput").ap()
    wproj_d = nc.dram_tensor("wproj", [4 * VD, DIM], f32r, kind="ExternalInput").ap()
    vones_d = nc.dram_tensor("vones", [P, NKT * 4], f32r, kind="ExternalInput").ap()
    neglam_d = nc.dram_tensor("neglam", [1, 1], f32, kind="ExternalInput").ap()
    out_d = nc.dram_tensor("outp", [N_TOK, DIM], f32, kind="ExternalOutput").ap()
    dbg = {}
    if _CACHE.get("debug"):
        dbg["qk"] = nc.dram_tensor("dbg_qk", [P, 4 * QB], f32, kind="ExternalOutput").ap()
        dbg["vx"] = nc.dram_tensor("dbg_vx", [P, 4 * VD1], f32, kind="ExternalOutput").ap()
        dbg["et"] = nc.dram_tensor("dbg_et", [P, 2 * QB], f32, kind="ExternalOutput").ap()
        dbg["o1"] = nc.dram_tensor("dbg_o1", [VD1, 4 * QB], f32, kind="ExternalOutput").ap()
        dbg["ot"] = nc.dram_tensor("dbg_ot", [VD, 4 * QB], f32, kind="ExternalOutput").ap()
        dbg["r1"] = nc.dram_tensor("dbg_r1", [VD, 4 * QB], f32, kind="ExternalOutput").ap()
        dbg["rstd"] = nc.dram_tensor("dbg_rstd", [VD, 4 * QB], f32, kind="ExternalOutput").ap()

    with ExitStack() as ctx:
        tc = ctx.enter_context(tile.TileContext(nc))

        singles = ctx.enter_context(tc.tile_pool(name="singles", bufs=1))
        ps_s = ctx.enter_context(tc.tile_pool(name="ps_s", bufs=2, space="PSUM"))
        ps_o = ctx.enter_context(tc.tile_pool(name="ps_o", bufs=1, space="PSUM"))

        # qk tiles: [m][n] -> (x @ Wm)^T chunk, m in (q1, q2, k1, k2), n = tok/512
        qk_sb = [
            [singles.tile([P, QB], f32r, tag=f"qk{m}_{n}", name=f"qk{m}_{n}") for n in range(NQB)]
            for m in range(4)
        ]
        # v tiles per token-tile, with the denominator ones column appended
        vx_sb = [singles.tile([P, 4, VD1], f32r, tag=f"vx{t}", name=f"vx{t}") for t in range(NKT)]
        wp_sb = singles.tile([VD, 4, DIM], f32r, tag="wp")
        ones64 = singles.tile([VD, VD], f32r, tag="ones64")
        neglam_sb = singles.tile([1, 1], f32, tag="nl")
        eps_sb = singles.tile([VD, 1], f32, tag="eps")

        nc.vector.memset(eps_sb, EPS)

        # ---- stage 1: qkv projections (x/W pool released afterwards) ----
        # qk_sb[:, m, :] = (x @ Wm)^T for m in (q1, q2, k1, k2); v in token-major
        # layout with a ones column appended per head.
        with tc.tile_pool(name="ph1", bufs=1) as ph1:
            # per-k-tile loads so qkv matmuls can chase the x DMAs
            xT_t = xT_d.rearrange("(ko p) t -> ko p t", p=P)
            wq_t = wqkv_d.rearrange("(ko p) c -> ko p c", p=P)
            x_sb = []
            w_sb = []
            for k in range(NKD):
                wt = ph1.tile([P, 768], f32r, tag=f"w{k}", name=f"w{k}")
                nc.sync.dma_start(wt, wq_t[k])
                w_sb.append(wt)
                xt = ph1.tile([P, N_TOK], f32r, tag=f"x{k}", name=f"x{k}")
                nc.sync.dma_start(xt, xT_t[k])
                x_sb.append(xt)
            # small late loads - not needed until AV / projection
            for t in range(NKT):
                nc.sync.dma_start(
                    vx_sb[t][:, :, VD:VD1],
                    vones_d.rearrange("p (t j) -> p t j", j=4)[:, t, :].unsqueeze(2),
                )
            nc.sync.dma_start(wp_sb, wproj_d.rearrange("(j v) c -> v j c", v=VD))
            nc.sync.dma_start(ones64, vones_d[0:VD, 0:VD])
            nc.sync.dma_start(neglam_sb, neglam_d)

            # v first (its psum pool slot must be free before the first
            # AV accumulator is allocated), then q1/k1 (needed by group-1
            # attention immediately), then q2/k2.
            for i in range(NKT):
                po = ps_o.tile([P, 4 * QB], f32, tag="o")
                pv = po[:, :4 * VD]
                for k in range(NKD):
                    nc.tensor.matmul(
                        pv,
                        lhsT=x_sb[k][:, i * P:(i + 1) * P],
                        rhs=w_sb[k][:, 512:768],
                        start=(k == 0),
                        stop=(k == NKD - 1),
                    )
                nc.vector.tensor_copy(
                    vx_sb[i][:, :, 0:VD], pv.rearrange("p (j v) -> p j v", j=4)
                )
            for n in range(NQB):
                for m in (0, 2, 1, 3):
                    ps = ps_s.tile([P, 2 * QB], f32, tag="s")
                    pqk = ps[:, :QB]
                    for k in range(NKD):
                        nc.tensor.matmul(
                            pqk,
                            lhsT=w_sb[k][:, m * P:(m + 1) * P],
                            rhs=x_sb[k][:, n * QB:(n + 1) * QB],
                            start=(k == 0),
                            stop=(k == NKD - 1),
                        )
                    nc.vector.tensor_copy(qk_sb[m][n], pqk)

        if dbg:
            dq = singles.tile([P, 4 * QB], f32, tag="dbgq")
            for m in range(4):
                nc.vector.tensor_copy(dq[:, m * QB:(m + 1) * QB], qk_sb[m][0])
            nc.sync.dma_start(dbg["qk"], dq)
            dv = singles.tile([P, 4 * VD1], f32, tag="dbgv")
            nc.vector.tensor_copy(dv.rearrange("p (j v) -> p j v", j=4), vx_sb[0])
            nc.sync.dma_start(dbg["vx"], dv)

        # ---- stage 2+3 pools (reuse the released x/W space) ----
        expp = ctx.enter_context(tc.tile_pool(name="expp", bufs=14))
        bcast = ctx.enter_context(tc.tile_pool(name="bcast", bufs=4))
        stage = ctx.enter_context(tc.tile_pool(name="stage", bufs=4))
        ocp = ctx.enter_context(tc.tile_pool(name="ocp", bufs=2))
        owk = ctx.enter_context(tc.tile_pool(name="owk", bufs=2))
        owk1 = ctx.enter_context(tc.tile_pool(name="owk1", bufs=1))

        def emit_proj_half(o_t, qb, half):
            """Partial projection for tok tiles (2*half, 2*half+1) of q-block qb."""
            q0 = qb * QB
            pp = ps_o.tile([P, 4 * QB], f32, tag="o")
            for sl in range(4):
                t = half * 2 + sl // 2
                nck = sl % 2
                outsl = pp[:, sl * QB:(sl + 1) * QB]
                for j in range(4):
                    nc.tensor.matmul(
                        outsl,
                        lhsT=o_t[:, j * QB + t * P:j * QB + (t + 1) * P],
                        rhs=wp_sb[:, j, nck * QB:(nck + 1) * QB],
                        start=(j == 0),
                        stop=(j == 3),
                        skip_group_check=True,
                    )
                st = stage.tile([P, QB], f32, tag="st")
                nc.vector.tensor_copy(st, outsl)
                nc.sync.dma_start(
                    out_d[q0 + t * P:q0 + (t + 1) * P, nck * QB:(nck + 1) * QB], st
                )

        def emit_combine_chunk(o1s, o2s, o_t, lo, hi):
            """Normalize both groups, differential combine, RMSNorm for
            columns [lo:hi) (contiguous sub-head blocks). Pure
            SBUF/DVE/Pool/ACT - no PSUM, so it overlaps attention freely."""
            w = hi - lo
            # DVE lanes are per-partition: move the denominator row (partition
            # VD) to partition 0 via DMA before computing reciprocals.
            r1b = bcast.tile([VD, 4 * QB], f32, tag="b", name="r1b")[:, :w]
            nc.sync.dma_start(r1b[0:1, :], o1s[VD:VD1, lo:hi])
            nc.vector.reciprocal_approx_fast(r1b[0:1, :], r1b[0:1, :])
            nc.gpsimd.partition_broadcast(r1b, r1b[0:1, :])
            r2b = bcast.tile([VD, 4 * QB], f32, tag="b", name="r2b")[:, :w]
            nc.sync.dma_start(r2b[0:1, :], o2s[VD:VD1, lo:hi])
            nc.vector.reciprocal_approx_fast(r2b[0:1, :], r2b[0:1, :])
            nc.vector.tensor_scalar_mul(r2b[0:1, :], r2b[0:1, :], neglam_sb[0:1, 0:1])
            nc.gpsimd.partition_broadcast(r2b, r2b[0:1, :])

            # o = attn1 - lam * attn2  (written as f32r for the projection)
            ot = o_t[:, lo:hi]
            sq_t = owk1.tile([VD, 4 * QB], f32, tag="sq", name="sq_t")[:, :w]
            nc.vector.tensor_mul(ot, o1s[0:VD, lo:hi], r1b)
            nc.vector.tensor_mul(r2b, o2s[0:VD, lo:hi], r2b)
            nc.vector.tensor_add(ot, ot, r2b)

            # RMSNorm over vd: ms broadcast to all rows via partition_all_reduce
            nc.vector.tensor_mul(sq_t, ot, ot)
            ssqb = bcast.tile([VD, 4 * QB], f32, tag="b", name="ssqb")[:, :w]
            nc.gpsimd.partition_all_reduce(ssqb, sq_t, VD, bass_isa.ReduceOp.add)
            rstd_t = bcast.tile([VD, 4 * QB], f32, tag="b", name="rstd_t")[:, :w]
            nc.scalar.activation(rstd_t, ssqb, AF.Ln, bias=eps_sb, scale=1.0 / VD)
            rstd = bcast.tile([VD, 4 * QB], f32, tag="b", name="rstd")[:, :w]
            nc.scalar.activation(rstd, rstd_t, AF.Exp, scale=-0.5)
            nc.vector.tensor_mul(ot, ot, rstd)

        def emit_combine(o1s, o2s, nchunks=1):
            o_t = owk.tile([VD, 4 * QB], f32r, tag="o", name="o_t")
            step = 4 * QB // nchunks
            for c in range(nchunks):
                emit_combine_chunk(o1s, o2s, o_t, c * step, (c + 1) * step)
            return o_t

        # ---- attention, with combine and projection of q-block N software-
        # pipelined into q-block N+1's group-1 attention ----
        prevo = None  # (o1s, o2s, qb) awaiting combine
        prev = None   # (o tile, qb) whose projection is still pending
        for qb in range(NQB):
            q0 = qb * QB
            og_sb = []
            for g in range(2):
                po = ps_o.tile([VD1, 4 * QB], f32, tag="o")
                for kt in range(NKT):
                    if g == 0 and kt == 2 and prevo is not None:
                        prev = (emit_combine(prevo[0], prevo[1]), prevo[2])
                        prevo = None
                    for h in range(2):
                        ps = ps_s.tile([P, 2 * QB], f32, tag="s")
                        for jj in range(2):
                            j = 2 * h + jj
                            # S^T[kt-block, q-block] for sub-head j of group g
                            nc.tensor.matmul(
                                ps[:, jj * QB:(jj + 1) * QB],
                                lhsT=qk_sb[2 + g][kt // NQB][
                                    32 * j:32 * (j + 1),
                                    (kt % NQB) * P:(kt % NQB + 1) * P,
                                ],
                                rhs=qk_sb[g][qb][32 * j:32 * (j + 1), :],
                                start=True,
                                stop=True,
                                tile_position=(32 * j, 0),
                            )
                        et = expp.tile([P, 2 * QB], f32r, tag="e")
                        nc.scalar.activation(et, ps, AF.Exp)
                        if dbg and qb == 0 and g == 0 and kt == 0 and h == 0:
                            de = singles.tile([P, 2 * QB], f32, tag="dbge")
                            nc.vector.tensor_copy(de, et)
                            nc.sync.dma_start(dbg["et"], de)
                        for jj in range(2):
                            j = 2 * h + jj
                            nc.tensor.matmul(
                                po[:, j * QB:(j + 1) * QB],
                                lhsT=vx_sb[kt][:, j, :],
                                rhs=et[:, jj * QB:(jj + 1) * QB],
                                start=(kt == 0),
                                stop=(kt == NKT - 1),
                                skip_group_check=True,
                            )
                og = ocp.tile([VD1, 4 * QB], f32, tag="og")
                nc.vector.tensor_copy(og, po)
                og_sb.append(og)
                if qb == NQB - 1 and g == 0:
                    # final block: overlap the group-1 normalizer with group-2
                    o_last = owk.tile([VD, 4 * QB], f32r, tag="o", name="o_last")
                    r1b_l = bcast.tile([VD, 4 * QB], f32, tag="b", name="r1b_l")
                    nc.sync.dma_start(r1b_l[0:1, :], og[VD:VD1, :])
                    nc.vector.reciprocal_approx_fast(r1b_l[0:1, :], r1b_l[0:1, :])
                    nc.gpsimd.partition_broadcast(r1b_l, r1b_l[0:1, :])
                    nc.vector.tensor_mul(o_last, og[0:VD, :], r1b_l)
                if dbg and qb == 0 and g == 0:
                    nc.sync.dma_start(dbg["o1"], og)
                if g == 0 and prev is not None:
                    emit_proj_half(*prev, half=0)
            if prev is not None:
                emit_proj_half(*prev, half=1)
                prev = None

            prevo = (og_sb[0], og_sb[1], qb)

        # tail: group-2 normalizer, differential combine, RMS, projection for
        # the final q-block (its group-1 half was computed during group 2)
        o2s = prevo[1]
        r2b_l = bcast.tile([VD, 4 * QB], f32, tag="b", name="r2b_l")
        nc.sync.dma_start(r2b_l[0:1, :], o2s[VD:VD1, :])
        nc.vector.reciprocal_approx_fast(r2b_l[0:1, :], r2b_l[0:1, :])
        nc.vector.tensor_scalar_mul(r2b_l[0:1, :], r2b_l[0:1, :], neglam_sb[0:1, 0:1])
        nc.gpsimd.partition_broadcast(r2b_l, r2b_l[0:1, :])
        nc.vector.tensor_mul(r2b_l, o2s[0:VD, :], r2b_l)
        nc.vector.tensor_add(o_last, o_last, r2b_l)
        sq_l = owk1.tile([VD, 4 * QB], f32r, tag="sq", name="sq_l")
        nc.vector.tensor_mul(sq_l, o_last, o_last)
        # RMS partition-reduce on the (idle) PE via ones-matmul, in the free
        # double-buffered score-psum slots; ln/exp per 2-chunk
        rstd_tl = bcast.tile([VD, 4 * QB], f32, tag="b", name="rstd_tl")
        rstd_l = bcast.tile([VD, 4 * QB], f32, tag="b", name="rstd_l")
        for c in range(2):
            pq = ps_s.tile([P, 2 * QB], f32, tag="s", name=f"ssqp{c}")
            for cc in range(2):
                nc.tensor.matmul(
                    pq[0:VD, cc * QB:(cc + 1) * QB],
                    lhsT=ones64,
                    rhs=sq_l[:, (2 * c + cc) * QB:(2 * c + cc + 1) * QB],
                    start=True,
                    stop=True,
                )
            sl = slice(2 * c * QB, (2 * c + 2) * QB)
            nc.scalar.activation(
                rstd_tl[:, sl], pq[0:VD, :], AF.Ln, bias=eps_sb, scale=1.0 / VD
            )
            nc.scalar.activation(rstd_l[:, sl], rstd_tl[:, sl], AF.Exp, scale=-0.5)
        nc.vector.tensor_mul(o_last, o_last, rstd_l)
        # final projection as 4 pipelined quarters on the score-psum slots
        for t in range(4):
            pq = ps_s.tile([P, 2 * QB], f32, tag="s", name=f"projq{t}")
            for nck in range(2):
                outsl = pq[:, nck * QB:(nck + 1) * QB]
                for j in range(4):
                    nc.tensor.matmul(
                        outsl,
                        lhsT=o_last[:, j * QB + t * P:j * QB + (t + 1) * P],
                        rhs=wp_sb[:, j, nck * QB:(nck + 1) * QB],
                        start=(j == 0),
                        stop=(j == 3),
                        skip_group_check=True,
                    )
                st = stage.tile([P, QB], f32, tag="st", name=f"stq{t}_{nck}")
                nc.vector.tensor_copy(st, outsl)
                nc.sync.dma_start(
                    out_d[
                        prevo[2] * QB + t * P:prevo[2] * QB + (t + 1) * P,
                        nck * QB:(nck + 1) * QB,
                    ],
                    st,
                )

    nc.compile()
    return nc


def _get_module():
    if "nc" not in _CACHE:
        _CACHE["nc"] = _build_module()
    return _CACHE["nc"]


def make_in_maps(inputs: dict) -> list:
    x = np.asarray(inputs["x"], np.float32)
    wqkv = np.asarray(inputs["W_qkv"], np.float32)
    wproj = np.asarray(inputs["W_proj"], np.float32)
    lq1 = np.asarray(inputs["lambda_q1"], np.float32)
    lk1 = np.asarray(inputs["lambda_k1"], np.float32)
    lq2 = np.asarray(inputs["lambda_q2"], np.float32)
    lk2 = np.asarray(inputs["lambda_k2"], np.float32)
    subw = np.asarray(inputs["subln_w"], np.float32)

    lam = float(
        np.exp(np.sum(lq1 * lk1)) - np.exp(np.sum(lq2 * lk2)) + LAMBDA_INIT
    )
    neglam = np.array([[-lam]], np.float32)
    vones = np.ones((P, NKT * 4), np.float32)
    wp_rowscale = (np.tile(subw, 4) * (1.0 - LAMBDA_INIT)).astype(np.float32)

    in_maps = []
    for c in range(NCORES):
        b, g = divmod(c, 4)
        xT = np.ascontiguousarray(x[b].T).astype(np.float32)
        ws = np.ascontiguousarray(
            np.concatenate(
                [
                    wqkv[:, 128 * g:128 * g + 128] * SCALE,
                    wqkv[:, 512 + 128 * g:512 + 128 * g + 128] * SCALE,
                    wqkv[:, 1024 + 128 * g:1024 + 128 * g + 128],
                    wqkv[:, 1536 + 128 * g:1536 + 128 * g + 128],
                    wqkv[:, 2048 + 256 * g:2048 + 256 * g + 256],
                ],
                axis=1,
            )
        ).astype(np.float32)
        wp = np.ascontiguousarray(
            wproj[256 * g:256 * (g + 1), :] * wp_rowscale[:, None]
        ).astype(np.float32)
        in_maps.append(
            {"xt": xT, "wqkv": ws, "wproj": wp, "neglam": neglam, "vones": vones}
        )
    return in_maps


def combine_outputs(inputs: dict, parts: list) -> np.ndarray:
    bproj = np.asarray(inputs["b_proj"], np.float32)
    out = np.stack(
        [
            parts[0] + parts[1] + parts[2] + parts[3],
            parts[4] + parts[5] + parts[6] + parts[7],
        ]
    )
    return (out + bproj[None, None, :]).astype(np.float32)


def kernel(**inputs) -> np.ndarray:
    from concourse import bass_utils

    nc = _get_module()
    in_maps = make_in_maps(inputs)
    res = bass_utils.run_bass_kernel_spmd(nc, in_maps, core_ids=list(range(NCORES)))
    parts = [np.asarray(res.results[c]["outp"], np.float32) for c in range(NCORES)]
    return combine_outputs(inputs, parts)
